# revision 3
# baseline (speedup 1.0000x reference)
import os
import sys
sys.path.insert(0, '/opt/trn_rl_repo')
import numpy as np
import concourse.bass as bass
import concourse.bacc as bacc
import concourse.mybir as mybir
import concourse.tile as tile
from concourse.bass import IndirectOffsetOnAxis
from concourse.bass_utils import run_bass_kernel_spmd

P = 128
T = 1024
S = 1024
D = 512
H = 8
DK = 64
DFF = 2048
VOC = 32000
NT = T // P   # 8 token tiles
ND = D // P   # 4 d-model chunks
NJ = DFF // P  # 16 dff tiles
L_FULL = 6
EPS = 1e-5

F32 = mybir.dt.float32
F32R = mybir.dt.float32r
BF16 = mybir.dt.bfloat16
I32 = mybir.dt.int32
AF = mybir.ActivationFunctionType
OP = mybir.AluOpType


def _pe_table():
    pos = np.arange(T)[:, None].astype(np.float64)
    div = np.exp(np.arange(0, D, 2).astype(np.float64) * (-np.log(10000.0) / D))
    pe = np.zeros((T, D))
    pe[:, 0::2] = np.sin(pos * div)
    pe[:, 1::2] = np.cos(pos * div)
    return pe.astype(np.float32)


def _causal_quads_bin():
    # [P, 4, 512] binary: 1.0 where key (128*r + kk) <= query qq else 0
    kk = np.arange(P)[:, None]
    qq = np.arange(512)[None, :]
    out = np.zeros((P, 4, 512), np.float32)
    for r in range(4):
        out[:, r, :] = np.where(128 * r + kk > qq, 0.0, 1.0)
    return out


def _unified_act_tables(arch):
    # Steer the act-table placement pass to natural_log_exp_and_others,
    # which serves every function this kernel uses (Exp/Ln/Relu/Copy) —
    # the greedy first-match choice would thrash between exp and ln sets.
    tabs = _real_get_activation_tables(arch)
    ours = {AF.Exp, AF.Ln, AF.Relu, AF.Copy, AF.Identity}
    out = {}
    for name, fns in tabs.items():
        if name == "natural_log_exp_and_others":
            out[name] = fns
        else:
            out[name] = fns - ours
    return out


_real_get_activation_tables = bacc.get_activation_tables


def build(n_layers=L_FULL):
    nc = bacc.Bacc("TRN2", target_bir_lowering=False, debug=False, num_devices=8)
    bacc.get_activation_tables = _unified_act_tables

    embd = nc.dram_tensor("emb", [VOC, D], F32, kind="ExternalInput")
    decd = nc.dram_tensor("dec_idx", [P, NT], I32, kind="ExternalInput")
    encd = nc.dram_tensor("enc_idx", [P, NT], I32, kind="ExternalInput")
    ped = nc.dram_tensor("pe", [T, D], F32, kind="ExternalInput")
    causd = nc.dram_tensor("causb", [P, 4, 512], BF16, kind="ExternalInput")
    identrd = nc.dram_tensor("identr", [P, P], F32R, kind="ExternalInput")
    encxd = nc.dram_tensor("encx", [S, D], F32, kind="ExternalInput")
    WQS = nc.dram_tensor("wq_s", [n_layers, D, D], F32R, kind="ExternalInput")
    WKS = nc.dram_tensor("wk_s", [n_layers, D, D], F32R, kind="ExternalInput")
    WVS = nc.dram_tensor("wv_s", [n_layers, D, D], F32R, kind="ExternalInput")
    WOS = nc.dram_tensor("wo_s", [n_layers, D, D], F32R, kind="ExternalInput")
    WQC = nc.dram_tensor("wq_c", [n_layers, D, D], F32R, kind="ExternalInput")
    WKC = nc.dram_tensor("wk_c", [n_layers, D, D], F32R, kind="ExternalInput")
    WVC = nc.dram_tensor("wv_c", [n_layers, D, D], F32R, kind="ExternalInput")
    WOC = nc.dram_tensor("wo_c", [n_layers, D, D], F32R, kind="ExternalInput")
    W1D = nc.dram_tensor("w1", [n_layers, D, DFF], F32R, kind="ExternalInput")
    W2D = nc.dram_tensor("w2", [n_layers, DFF, D], F32R, kind="ExternalInput")
    outd = nc.dram_tensor("out", [T, D], F32R, kind="ExternalOutput")

    with nc.allow_low_precision(reason="f32r/bf16 rounding intended"), \
         tile.TileContext(nc) as tc:
        with tc.tile_pool(name="pers", bufs=1) as pers, \
             tc.tile_pool(name="watt", bufs=1) as watt, \
             tc.tile_pool(name="wff", bufs=1) as wff, \
             tc.tile_pool(name="w2p", bufs=2) as w2p, \
             tc.tile_pool(name="pe3", bufs=4) as pe3, \
             tc.tile_pool(name="p3", bufs=3) as p3, \
             tc.tile_pool(name="p4", bufs=4) as p4, \
             tc.tile_pool(name="prb", bufs=2) as prb, \
             tc.tile_pool(name="pctx", bufs=4) as pctx, \
             tc.tile_pool(name="pce", bufs=8) as pce, \
             tc.tile_pool(name="psS", bufs=2, space="PSUM") as psS, \
             tc.tile_pool(name="psD", bufs=2, space="PSUM") as psD, \
             tc.tile_pool(name="psC", bufs=2, space="PSUM") as psC:

            # ---------------- persistent tiles ----------------
            x_res = pers.tile([P, NT, D], F32R)      # [tok_in_tile, t_tile, D]
            xT = pers.tile([P, ND, T], F32R)         # [d_in_chunk, d_chunk, tok]
            encT = pers.tile([P, ND, S], F32R)
            QT = pers.tile([P, ND, 512], F32R)       # current query half
            KT = pers.tile([P, ND, T], F32R)
            vext = pers.tile([P, NT, H, DK + 1], BF16)
            causal_sb = pers.tile([P, 4, 512], BF16)
            identr_sb = pers.tile([P, P], F32R)
            eps_sb = pers.tile([P, 1], F32)
            dec_sb = pers.tile([P, NT], I32)
            enc_sb = pers.tile([P, NT], I32)
            msc_dec = pers.tile([P, NT], F32)
            msc_enc = pers.tile([P, NT], F32)

            nc.sync.dma_start(out=causal_sb, in_=causd[:, :, :])
            nc.sync.dma_start(out=identr_sb, in_=identrd[:, :])
            nc.sync.dma_start(out=dec_sb, in_=decd[:, :])
            nc.sync.dma_start(out=enc_sb, in_=encd[:, :])
            nc.vector.memset(eps_sb, EPS)

            def transpose_to(dst, src_of_tt, tts):
                # dst [P, ND, T]; src_of_tt(tt) -> [P, D] AP (f32r for speed)
                for d in range(ND):
                    for g0 in range(0, len(tts), 4):
                        grp = tts[g0:g0 + 4]
                        ps_t = psD.tile([P, 512], F32R, tag="d",
                                        name=f"pst_{d}_{grp[0]}")
                        for qi, tt in enumerate(grp):
                            nc.tensor.matmul(
                                ps_t[:, qi * P:(qi + 1) * P],
                                src_of_tt(tt)[:, d * P:(d + 1) * P],
                                identr_sb, is_transpose=True,
                                start=(qi == 0), stop=(qi == 3),
                                skip_group_check=True)
                        nc.vector.tensor_copy(
                            out=dst[:, d, grp[0] * P:(grp[0] + 4) * P],
                            in_=ps_t)

            for g0 in range(0, NT, 4):
                e_ts = []
                for tt in range(g0, g0 + 4):
                    e_t = pctx.tile([P, D], F32R, tag="ctxp", name=f"enc_{tt}")
                    nc.gpsimd.dma_start(out=e_t,
                                        in_=encxd[tt * P:(tt + 1) * P, :])
                    e_ts.append(e_t)
                transpose_to(encT, lambda tt: e_ts[tt - g0],
                             list(range(g0, g0 + 4)))

            # pad multipliers (0 for pad token, 1 otherwise)
            for tok_sb, msc in ((dec_sb, msc_dec), (enc_sb, msc_enc)):
                tokf = p4.tile([P, NT], F32, tag="tokf")
                nc.vector.tensor_copy(out=tokf, in_=tok_sb)
                is0 = p4.tile([P, NT], F32, tag="is0")
                nc.vector.tensor_scalar(out=is0, in0=tokf, scalar1=0.0,
                                        scalar2=None, op0=OP.is_equal)
                nc.scalar.activation(out=msc, in_=is0, func=AF.Copy,
                                     bias=1.0, scale=-1.0)

            # ---------------- embedding + pe ----------------
            for tt in range(NT):
                g = p3.tile([P, D], F32, tag="tmp")
                nc.gpsimd.indirect_dma_start(
                    out=g, out_offset=None, in_=embd[:, :],
                    in_offset=IndirectOffsetOnAxis(ap=dec_sb[:, tt:tt + 1], axis=0))
                pe_t = p3.tile([P, D], F32, tag="tmp")
                nc.sync.dma_start(out=pe_t, in_=ped[tt * P:(tt + 1) * P, :])
                g2 = p3.tile([P, D], F32, tag="tmp")
                nc.vector.tensor_scalar(out=g2, in0=g,
                                        scalar1=msc_dec[:, tt:tt + 1],
                                        scalar2=None, op0=OP.mult)
                nc.vector.tensor_add(out=x_res[:, tt, :], in0=g2, in1=pe_t)

            transpose_to(xT, lambda tt: x_res[:, tt, :], list(range(NT)))

            # ---------------- helpers ----------------
            def ln_into_xres(ps_in, tt):
                # layernorm(ps_in + x_res[tt]) -> x_res[tt]
                pre = p3.tile([P, D], F32, tag="tmp")
                nc.vector.tensor_add(out=pre, in0=ps_in, in1=x_res[:, tt, :])
                st = p4.tile([P, nc.vector.BN_STATS_DIM], F32, tag="st")
                nc.vector.bn_stats(out=st, in_=pre)
                mv = p4.tile([P, nc.vector.BN_AGGR_DIM], F32, tag="mv")
                nc.vector.bn_aggr(out=mv, in_=st)
                # rstd = exp(-0.5*ln(var+eps)); stays in the exp act table
                lnv = p4.tile([P, 1], F32, tag="sd")
                nc.scalar.activation(out=lnv, in_=mv[:, 1:2], func=AF.Ln,
                                     bias=eps_sb, scale=1.0)
                rstd = p4.tile([P, 1], F32, tag="rs")
                nc.scalar.activation(out=rstd, in_=lnv, func=AF.Exp,
                                     scale=-0.5)
                nc.vector.tensor_scalar(out=x_res[:, tt, :], in0=pre,
                                        scalar1=mv[:, 0:1], scalar2=rstd,
                                        op0=OP.subtract, op1=OP.mult)

            def load_w(wd, l, tag):
                w = watt.tile([P, ND, D], F32R, tag=tag)
                nc.gpsimd.dma_start(
                    out=w, in_=wd[l].rearrange("(kc kp) n -> kp kc n", kp=P))
                return w

            def proj_qt(w, c):
                for dq in range(ND):
                    ps = psD.tile([P, 512], F32, tag="d")
                    for kc in range(ND):
                        nc.tensor.matmul(
                            ps, w[:, kc, dq * P:(dq + 1) * P],
                            xT[:, kc, c * 512:(c + 1) * 512],
                            start=(kc == 0), stop=(kc == ND - 1))
                    nc.scalar.activation(out=QT[:, dq, :], in_=ps,
                                         func=AF.Copy)

            def proj_kt(w, src):
                for dq in range(ND):
                    for c in range(2):
                        ps = psD.tile([P, 512], F32, tag="d")
                        for kc in range(ND):
                            nc.tensor.matmul(
                                ps, w[:, kc, dq * P:(dq + 1) * P],
                                src[:, kc, c * 512:(c + 1) * 512],
                                start=(kc == 0), stop=(kc == ND - 1))
                        nc.scalar.activation(
                            out=KT[:, dq, c * 512:(c + 1) * 512],
                            in_=ps, func=AF.Copy)

            def proj_v(w, src, msc, tiles):
                for i in tiles:
                    ps = psD.tile([P, 512], F32, tag="d")
                    for kc in range(ND):
                        nc.tensor.matmul(ps,
                                         src[:, kc, i * P:(i + 1) * P],
                                         w[:, kc, :],
                                         start=(kc == 0), stop=(kc == ND - 1))
                    nc.vector.tensor_scalar(
                        out=vext[:, i, :, 0:DK],
                        in0=ps.rearrange("p (h v) -> p h v", h=H),
                        scalar1=msc[:, i:i + 1], scalar2=None, op0=OP.mult)
                for h in range(H):
                    nc.vector.tensor_copy(
                        out=vext[:, :, h, DK:DK + 1].rearrange(
                            "p t o -> p (t o)"),
                        in_=msc[:, :])

            def attn_score(l, is_self, c, tag):
                # scores/exp/AV with AV lagging one step behind the score
                # matmuls (hides exp latency at pair starts). After each
                # pair's last AV the ctx PSUM is drained to SBUF in bf16 and
                # normalized there, so the 2 psC slots recycle quickly and
                # the normalize chain overlaps the next pair's scores.
                kmax = 4 * (c + 1) if is_self else NT
                ctxes = [[None, None] for _ in range(ND)]
                pending = None
                for d in range(ND):
                    ps_ctx = [psC.tile([P, 512], F32, tag="c",
                                       name=f"cx_{tag}_{c}_{d}_{hh}")
                              for hh in range(2)]
                    for i0 in range(0, kmax, 2):
                        diag = is_self and i0 >= 4 * c
                        es = [None, None]
                        for hh in range(2):
                            hsl = slice(hh * 64, (hh + 1) * 64)
                            ps_s = psS.tile([P, 1024], F32, tag="s")
                            for half, i in ((0, i0), (1, i0 + 1)):
                                sl_ = slice(half * 512, (half + 1) * 512)
                                nc.tensor.matmul(
                                    ps_s[:, sl_],
                                    KT[hsl, d, i * P:(i + 1) * P],
                                    QT[hsl, d, :],
                                    start=True, stop=True,
                                    skip_group_check=True)
                            e = pe3.tile([P, 1024], BF16, tag="exp")
                            nc.scalar.activation(out=e, in_=ps_s,
                                                 func=AF.Exp, scale=0.125)
                            es[hh] = e
                        if diag:
                            r = i0 - 4 * c
                            for hh in range(2):
                                nc.vector.tensor_mul(
                                    out=es[hh], in0=es[hh],
                                    in1=causal_sb[:, r:r + 2, :].rearrange(
                                        "p r q -> p (r q)"))
                        if pending is not None:
                            pending()

                        def make_av(d_=d, i0_=i0, es_=es, ctx_=ps_ctx):
                            for hh in range(2):
                                h = 2 * d_ + hh
                                for half, i in ((0, i0_), (1, i0_ + 1)):
                                    nc.tensor.matmul(
                                        ctx_[hh][0:DK + 1, :],
                                        vext[:, i, h, :],
                                        es_[hh][:, half * 512:(half + 1) * 512],
                                        start=(i == 0), stop=(i == kmax - 1))
                            if i0_ == kmax - 2:
                                for hh in range(2):
                                    ce = pce.tile([DK + 1, 512], BF16,
                                                   tag="ctxe",
                                                   name=f"ce_{tag}_{c}_{d_}_{hh}")
                                    nc.vector.tensor_copy(
                                        out=ce, in_=ctx_[hh][0:DK + 1, :])
                                    ctxes[d_][hh] = ce
                        pending = make_av
                if pending is not None:
                    pending()
                return ctxes

            def attn_finish(l, c, wo, tag, ctxes, transpose=True):
                ctx_pairs = [pctx.tile([P, 512], F32R, tag="ctxp",
                                       name=f"cp_{tag}_{c}_{d}")
                             for d in range(ND)]
                for d in range(ND):
                    for hh in range(2):
                        hsl = slice(hh * 64, (hh + 1) * 64)
                        ce = ctxes[d][hh]
                        recip = prb.tile([1, 512], BF16, tag="recip")
                        nc.vector.reciprocal(out=recip, in_=ce[DK:DK + 1, :])
                        rb = prb.tile([64, 512], BF16, tag="rb")
                        nc.gpsimd.partition_broadcast(rb, recip)
                        nc.vector.tensor_mul(out=ctx_pairs[d][hsl, :],
                                             in0=ce[0:DK, :], in1=rb)
                for ts_ in range(4):
                    tt = 4 * c + ts_
                    ps_o = psD.tile([P, 512], F32, tag="d")
                    for d in range(ND):
                        nc.tensor.matmul(
                            ps_o,
                            ctx_pairs[d][:, ts_ * P:(ts_ + 1) * P],
                            wo[:, d, :], start=(d == 0), stop=(d == ND - 1))
                    ln_into_xres(ps_o, tt)
                if transpose:
                    transpose_to(xT, lambda tt_: x_res[:, tt_, :],
                                 list(range(4 * c, 4 * c + 4)))

            def ffn(l, w1t, last):
                ps_fs = {}
                for c in range(2):
                    ps_f = [psS.tile([P, 1024], F32, tag="s",
                                     name=f"psf_{l}_{c}_{i}")
                            for i in range(2)]
                    ps_fs[c] = ps_f
                    for j in range(NJ):
                        if j % 2 == 0:
                            jc = j // 2
                            w2t = w2p.tile([P, 2, D], F32R, tag="w2c")
                            nc.gpsimd.dma_start(
                                out=w2t,
                                in_=W2D[l][jc * 256:(jc + 1) * 256, :]
                                .rearrange("(jj kp) n -> kp jj n", kp=P))
                        ps_h = psC.tile([P, 512], F32, tag="c")
                        for kc in range(ND):
                            nc.tensor.matmul(
                                ps_h, w1t[:, kc, j * P:(j + 1) * P],
                                xT[:, kc, c * 512:(c + 1) * 512],
                                start=(kc == 0), stop=(kc == ND - 1))
                        hT = p3.tile([P, 512], F32R, tag="hT")
                        nc.scalar.activation(out=hT, in_=ps_h, func=AF.Relu)
                        for ts_ in range(4):
                            nc.tensor.matmul(
                                ps_f[ts_ // 2][:, (ts_ % 2) * 512:
                                               (ts_ % 2 + 1) * 512],
                                hT[:, ts_ * P:(ts_ + 1) * P],
                                w2t[:, j % 2, :],
                                start=(j == 0), stop=(j == NJ - 1),
                                skip_group_check=True)
                    # LN for this half (c=1 matmuls above already fill c=0's
                    # LN tail since LN runs on DVE/Act)
                    for ts_ in range(4):
                        ln_into_xres(
                            ps_f[ts_ // 2][:, (ts_ % 2) * 512:
                                           (ts_ % 2 + 1) * 512],
                            4 * c + ts_)

            # ---------------- layers ----------------
            def transpose_half(c):
                transpose_to(xT, lambda tt: x_res[:, tt, :],
                             list(range(4 * c, 4 * c + 4)))

            for l in range(n_layers):
                # ---- self attention ----
                wk = load_w(WKS, l, "wk")
                wv = load_w(WVS, l, "wv")
                wq = load_w(WQS, l, "wq")
                wo = load_w(WOS, l, "wo")
                w1t = wff.tile([P, ND, DFF], F32R, tag="w1")
                nc.gpsimd.dma_start(
                    out=w1t,
                    in_=W1D[l].rearrange("(kc kp) n -> kp kc n", kp=P))
                proj_kt(wk, xT)
                proj_v(wv, xT, msc_dec, list(range(NT)))
                proj_qt(wq, 0)
                ctxs0 = attn_score(l, True, 0, f"{l}_s")
                proj_qt(wq, 1)                      # fills c0 tail
                attn_finish(l, 0, wo, f"{l}_s", ctxs0, transpose=False)
                ctxs1 = attn_score(l, True, 1, f"{l}_s")
                transpose_half(0)                   # runs during c1 tail
                # cross K/V projections also fill the self-c1 tail
                wk = load_w(WKC, l, "wk")
                wv = load_w(WVC, l, "wv")
                proj_kt(wk, encT)
                proj_v(wv, encT, msc_enc, list(range(NT)))
                attn_finish(l, 1, wo, f"{l}_s", ctxs1, transpose=False)
                # ---- cross attention ----
                wq = load_w(WQC, l, "wq")
                wo = load_w(WOC, l, "wo")
                proj_qt(wq, 0)                      # needs tiles 0-3 only
                transpose_half(1)                   # self tiles 4-7
                ctxs0 = attn_score(l, False, 0, f"{l}_c")
                proj_qt(wq, 1)
                attn_finish(l, 0, wo, f"{l}_c", ctxs0, transpose=False)
                ctxs1 = attn_score(l, False, 1, f"{l}_c")
                transpose_half(0)                   # cross tiles 0-3
                attn_finish(l, 1, wo, f"{l}_c", ctxs1, transpose=False)
                transpose_half(1)
                ffn(l, w1t, last=(l == n_layers - 1))
                if l < n_layers - 1:
                    transpose_half(0)
                    transpose_half(1)

            for tt in range(NT):
                nc.sync.dma_start(out=outd[tt * P:(tt + 1) * P, :],
                                  in_=x_res[:, tt, :])

    try:
        nc.compile()
    finally:
        bacc.get_activation_tables = _real_get_activation_tables
    return nc


_CACHE = {}


def get_nc(n_layers=L_FULL):
    if n_layers not in _CACHE:
        _CACHE[n_layers] = build(n_layers)
    return _CACHE[n_layers]


def make_in_maps(dec_inputs, enc_inputs, enc_outputs, emb,
                 Wq_self, Wk_self, Wv_self, Wo_self,
                 Wq_cross, Wk_cross, Wv_cross, Wo_cross, W1, W2,
                 n_layers=L_FULL):
    import ml_dtypes
    f = np.ascontiguousarray
    emb = f(np.asarray(emb, dtype=np.float32))
    dec = np.asarray(dec_inputs).astype(np.int32)
    enc = np.asarray(enc_inputs).astype(np.int32)
    encx = np.asarray(enc_outputs, dtype=np.float32)
    pe = _pe_table()
    causb = _causal_quads_bin().astype(ml_dtypes.bfloat16)
    identr = np.eye(P, dtype=np.float32)
    ws = {}
    for name, w in (("wq_s", Wq_self), ("wk_s", Wk_self), ("wv_s", Wv_self),
                    ("wo_s", Wo_self), ("wq_c", Wq_cross), ("wk_c", Wk_cross),
                    ("wv_c", Wv_cross), ("wo_c", Wo_cross), ("w1", W1),
                    ("w2", W2)):
        ws[name] = f(np.asarray(w, dtype=np.float32)[:n_layers])
    B = dec.shape[0]
    in_maps = []
    for b in range(B):
        m = dict(emb=emb, pe=pe, causb=causb, identr=identr,
                 dec_idx=f(dec[b].reshape(NT, P).T),
                 enc_idx=f(enc[b].reshape(NT, P).T),
                 encx=f(encx[b]), **ws)
        in_maps.append(m)
    return in_maps


def kernel(**inputs):
    n_layers = inputs.pop("_n_layers", L_FULL)
    nc = get_nc(n_layers)
    in_maps = make_in_maps(**inputs, n_layers=n_layers)
    res = run_bass_kernel_spmd(nc, in_maps, core_ids=list(range(len(in_maps))))
    out = np.stack([np.asarray(r["out"], dtype=np.float32)
                    for r in res.results], axis=0)
    return out
